# revision 1
# baseline (speedup 1.0000x reference)
"""CrossCCC loss kernel for Trainium2 (8 NeuronCores, sequence-parallel).

Math
----
reference computes, for lags n = 0..249:
    pred_n = [n zeros] ++ prediction[:T-n]
    ccc_n  = 2*cov(pred_n, gt) / (var_gt + var_pred_n + (mean_gt - mean_pred_n)^2)
    out    = 1 - mean_n(ccc_n)

Every lag statistic decomposes into lag-independent global sums plus tiny
suffix corrections:
    sum_n   = S_p - R_n          (R_n = sum of last n elements of p)
    sumsq_n = Q_p - R2_n
    cross_n = X_n = sum_j p[j] * gt[j+n]        (raw cross-correlation)
    cov_n   = (X_n - mean_gt*sum_n - mean_pred_n*Sv) / T,   Sv = S_g - T*mean_gt

The only heavy term is X_n for 250 lags.  With j = 128*a + k:
    X_n = sum_k G[k, k+n],   G[k, s] = sum_a p[128a + k] * gt[128a + s]
for s in [0, 384).  G is a Gram-style matmul with contraction over the long
block axis `a` -> runs on the tensor engine at full tile size.

Sharding: `a` blocks are split across 8 cores (sequence parallel).  Each core
views its 131072-element p shard as [128, 1024] row-major (row q covers
blocks 8q..8q+7) and its gt shard (+256 halo) as overlapping rows [128, 1280].
Then for column-tile t in 0..7:
    lhsT = p2d[:, 128t : 128t+128]    (K=128 rows q, M=128 lanes k)
    rhs  = gt2d[:, 128t : 128t+384]   (K=128 rows q, N=384 lanes s)
    G   += lhsT.T @ rhs               (PSUM accumulate)
Per-core partial sums of p, p^2, g, g^2 ride along on the vector/scalar
engines.  Host sums the 8 partial G's, takes 250 diagonal traces, and
finishes the scalar formula in float64.
"""

import numpy as np

T = 1_000_000
N_CORES = 8
ROWS = 128          # SBUF partitions; also the k-lane count
COLS = 1024         # per-row elements = 8 column-tiles of 128
SHARD = ROWS * COLS  # 131072 elements of p per core
HALO = 256           # gt halo: max lag reach 249 rounded to 2 blocks
GCOLS = COLS + HALO  # 1280
NTILES = COLS // 128  # 8
NS = 384             # rhs free size: covers s = k + n, n<250, k<128
NLAGS = 250
OUTC = NS + 4        # packed output: [G | sum_p | sumsq_p | sum_g | sumsq_g]

_compiled = None


def _build():
    import concourse.bacc as bacc
    import concourse.mybir as mybir
    import concourse.tile as tile

    f32 = mybir.dt.float32
    nc = bacc.Bacc("TRN2", target_bir_lowering=False, debug=False)

    p_dram = nc.dram_tensor("p", [ROWS, COLS], f32, kind="ExternalInput")
    g_dram = nc.dram_tensor("g", [ROWS, GCOLS], f32, kind="ExternalInput")
    out_dram = nc.dram_tensor("out", [ROWS, OUTC], f32, kind="ExternalOutput")

    with tile.TileContext(nc) as tc:
        with (
            tc.tile_pool(name="io", bufs=1) as io_pool,
            tc.tile_pool(name="scratch", bufs=1) as scratch_pool,
            tc.tile_pool(name="psum", bufs=1, space="PSUM") as psum_pool,
        ):
            p_sb = io_pool.tile([ROWS, COLS], f32)
            g_sb = io_pool.tile([ROWS, GCOLS], f32)
            out_sb = io_pool.tile([ROWS, OUTC], f32)

            nc.sync.dma_start(p_sb[:], p_dram[:])
            nc.sync.dma_start(g_sb[:], g_dram[:])

            gram = psum_pool.tile([ROWS, NS], f32)
            for t in range(NTILES):
                nc.tensor.matmul(
                    gram[:],
                    p_sb[:, t * 128 : t * 128 + 128],
                    g_sb[:, t * 128 : t * 128 + NS],
                    start=(t == 0),
                    stop=(t == NTILES - 1),
                )

            # per-partition scalar partials
            sq = scratch_pool.tile([ROWS, COLS], f32)
            nc.vector.reduce_sum(
                out_sb[:, NS : NS + 1], p_sb[:], axis=mybir.AxisListType.X
            )
            nc.scalar.activation(
                sq[:],
                p_sb[:],
                mybir.ActivationFunctionType.Square,
                accum_out=out_sb[:, NS + 1 : NS + 2],
            )
            nc.vector.reduce_sum(
                out_sb[:, NS + 2 : NS + 3],
                g_sb[:, :COLS],
                axis=mybir.AxisListType.X,
            )
            nc.scalar.activation(
                sq[:],
                g_sb[:, :COLS],
                mybir.ActivationFunctionType.Square,
                accum_out=out_sb[:, NS + 3 : NS + 4],
            )

            nc.vector.tensor_copy(out_sb[:, :NS], gram[:])
            nc.sync.dma_start(out_dram[:], out_sb[:])

    nc.compile()
    return nc


def _get_compiled():
    global _compiled
    if _compiled is None:
        _compiled = _build()
    return _compiled


def _shard_inputs(p: np.ndarray, g: np.ndarray):
    p_pad = np.zeros(N_CORES * SHARD, np.float32)
    p_pad[:T] = p
    g_pad = np.zeros(N_CORES * SHARD + HALO, np.float32)
    g_pad[:T] = g
    in_maps = []
    for c in range(N_CORES):
        p2d = p_pad[c * SHARD : (c + 1) * SHARD].reshape(ROWS, COLS)
        base = g_pad[c * SHARD : (c + 1) * SHARD + HALO]
        g2d = np.lib.stride_tricks.as_strided(
            base, shape=(ROWS, GCOLS), strides=(4 * COLS, 4)
        )
        in_maps.append(
            {"p": np.ascontiguousarray(p2d), "g": np.ascontiguousarray(g2d)}
        )
    return in_maps


def _finish(results, p: np.ndarray):
    """Small all-reduce over the 250-lag statistics, in float64."""
    G = np.zeros((ROWS, NS), np.float64)
    S_p = Q_p = S_g = Q_g = 0.0
    for r in results:
        o = r["out"].astype(np.float64)
        G += o[:, :NS]
        S_p += o[:, NS].sum()
        Q_p += o[:, NS + 1].sum()
        S_g += o[:, NS + 2].sum()
        Q_g += o[:, NS + 3].sum()

    X = np.array([np.trace(G, offset=n) for n in range(NLAGS)])

    p64 = p.astype(np.float64)
    tail = p64[T - NLAGS + 1 :][::-1]  # last 249 elements, reversed
    R = np.concatenate([[0.0], np.cumsum(tail)])        # R[n], n=0..249
    R2 = np.concatenate([[0.0], np.cumsum(tail * tail)])

    m = S_g / T
    var_g = (Q_g - T * m * m) / (T - 1)
    Sv = S_g - T * m

    n = np.arange(NLAGS)
    sum_n = S_p - R
    mp = sum_n / T
    sumsq_n = Q_p - R2
    var_p = (sumsq_n - T * mp * mp) / (T - 1)
    cov = (X - m * sum_n - mp * Sv) / T
    denom = var_g + var_p + (m - mp) ** 2
    ccc = 2.0 * cov / denom
    return np.float32(1.0 - ccc.mean())


def kernel(prediction: np.ndarray, ground_truth: np.ndarray) -> np.ndarray:
    from concourse import bass_utils

    p = np.asarray(prediction, np.float32).reshape(-1)
    g = np.asarray(ground_truth, np.float32).reshape(-1)
    assert p.shape == (T,) and g.shape == (T,)

    nc = _get_compiled()
    in_maps = _shard_inputs(p, g)
    res = bass_utils.run_bass_kernel_spmd(nc, in_maps, core_ids=list(range(N_CORES)))
    return _finish(res.results, p)


# revision 2
# speedup vs baseline: 1.2467x; 1.2467x over previous
"""CrossCCC loss kernel for Trainium2 (8 NeuronCores, sequence-parallel).

Math
----
reference computes, for lags n = 0..249:
    pred_n = [n zeros] ++ prediction[:T-n]
    ccc_n  = 2*cov(pred_n, gt) / (var_gt + var_pred_n + (mean_gt - mean_pred_n)^2)
    out    = 1 - mean_n(ccc_n)

Every lag statistic decomposes into lag-independent global sums plus tiny
suffix corrections:
    sum_n   = S_p - R_n          (R_n = sum of last n elements of p)
    sumsq_n = Q_p - R2_n
    cov_n   = (X_n - mean_gt*sum_n - mean_pred_n*Sv) / T,   Sv = S_g - T*mean_gt
with X_n = sum_j p[j]*gt[j+n] the raw cross-correlation -- the only heavy
term.  With j = 128*a + k:
    X_n = sum_k G[k, k+n],   G[k, s] = sum_a p[128a + k] * gt[128a + s]
for s in [0, 384): a Gram-style matmul contracting over the long block axis.
Q_p likewise falls out of trace(PP), PP[k,k'] = sum_a p[128a+k]*p[128a+k'].

Sharding: blocks are split across 8 cores.  Each core views its
131072-element p shard as [128, 1024] row-major (row q covers blocks
8q..8q+7) and its gt shard (+256 halo) as overlapping rows [128, 1280], so
for column-tile t in 0..7:
    G  += p2d[:, 128t:128t+128].T @ gt2d[:, 128t:128t+384]   (PSUM accumulate)
    PP += p2d[:, 128t:128t+128].T @ p2d[:, 128t:128t+128]
Inputs are pre-cast to bf16 on the host (exact bf16 products accumulate in
fp32 PSUM; final result error ~1e-6 relative).  Per-core partial sums of p,
g, g^2 ride along on the vector/scalar engines.  Host sums the 8 partial
G/PP's, takes diagonal traces, and finishes the scalar formula in float64.
"""

import numpy as np

T = 1_000_000
N_CORES = 8
ROWS = 128          # SBUF partitions; also the k-lane count
COLS = 1024         # per-row elements = 8 column-tiles of 128
SHARD = ROWS * COLS  # 131072 elements of p per core
HALO = 256           # gt halo: max lag reach 249 rounded to 2 blocks
GCOLS = COLS + HALO  # 1280
NTILES = COLS // 128  # 8
NS = 384             # G free size: covers s = k + n, n<250, k<128
NLAGS = 250
OUT1C = NS + 128     # bf16 matmul results: [G | PP]
OUT2C = 3            # f32 partial sums: [sum_p | sum_g | sumsq_g]

_compiled = None


def _build():
    import concourse.bacc as bacc
    import concourse.mybir as mybir
    import concourse.tile as tile

    f32 = mybir.dt.float32
    bf16 = mybir.dt.bfloat16
    nc = bacc.Bacc("TRN2", target_bir_lowering=False, debug=False)

    p_dram = nc.dram_tensor("p", [ROWS, COLS], bf16, kind="ExternalInput")
    g_dram = nc.dram_tensor("g", [ROWS, GCOLS], bf16, kind="ExternalInput")
    out1_dram = nc.dram_tensor("out1", [ROWS, OUT1C], bf16, kind="ExternalOutput")
    out2_dram = nc.dram_tensor("out2", [ROWS, OUT2C], f32, kind="ExternalOutput")

    with tile.TileContext(nc) as tc:
        with (
            tc.tile_pool(name="io", bufs=1) as io_pool,
            tc.tile_pool(name="scratch", bufs=1) as scratch_pool,
            tc.tile_pool(name="psum", bufs=1, space="PSUM") as psum_pool,
        ):
            pb = io_pool.tile([ROWS, COLS], bf16)
            gb = io_pool.tile([ROWS, GCOLS], bf16)
            out1 = io_pool.tile([ROWS, OUT1C], bf16)
            out2 = io_pool.tile([ROWS, OUT2C], f32)

            # split loads across both HWDGE rings so tile-0 operands land early
            nc.sync.dma_start(gb[:, 0:512], g_dram[:, 0:512])
            nc.sync.dma_start(pb[:], p_dram[:])
            nc.scalar.dma_start(gb[:, 512:GCOLS], g_dram[:, 512:GCOLS])

            gram = psum_pool.tile([ROWS, NS], f32)
            pp = psum_pool.tile([ROWS, 128], f32)
            for t in range(NTILES):
                lhsT = pb[:, t * 128 : t * 128 + 128]
                nc.tensor.matmul(
                    gram[:],
                    lhsT,
                    gb[:, t * 128 : t * 128 + NS],
                    start=(t == 0),
                    stop=(t == NTILES - 1),
                )
                nc.tensor.matmul(
                    pp[:],
                    lhsT,
                    pb[:, t * 128 : t * 128 + 128],
                    start=(t == 0),
                    stop=(t == NTILES - 1),
                )

            # per-partition scalar partials
            sq = scratch_pool.tile([ROWS, COLS], bf16)
            nc.vector.reduce_sum(out2[:, 0:1], pb[:], axis=mybir.AxisListType.X)
            nc.vector.reduce_sum(
                out2[:, 1:2], gb[:, :COLS], axis=mybir.AxisListType.X
            )
            nc.scalar.activation(
                sq[:],
                gb[:, :COLS],
                mybir.ActivationFunctionType.Square,
                accum_out=out2[:, 2:3],
            )

            nc.vector.tensor_copy(out1[:, :NS], gram[:])
            nc.vector.tensor_copy(out1[:, NS:OUT1C], pp[:])
            nc.sync.dma_start(out1_dram[:], out1[:])
            nc.scalar.dma_start(out2_dram[:], out2[:])

    nc.compile()
    return nc


def _get_compiled():
    global _compiled
    if _compiled is None:
        _compiled = _build()
    return _compiled


def _shard_inputs(p: np.ndarray, g: np.ndarray):
    import ml_dtypes

    bf = ml_dtypes.bfloat16
    p_pad = np.zeros(N_CORES * SHARD, bf)
    p_pad[:T] = p.astype(bf)
    g_pad = np.zeros(N_CORES * SHARD + HALO, bf)
    g_pad[:T] = g.astype(bf)
    in_maps = []
    for c in range(N_CORES):
        p2d = p_pad[c * SHARD : (c + 1) * SHARD].reshape(ROWS, COLS)
        base = g_pad[c * SHARD : (c + 1) * SHARD + HALO]
        g2d = np.lib.stride_tricks.as_strided(
            base, shape=(ROWS, GCOLS), strides=(2 * COLS, 2)
        )
        in_maps.append(
            {"p": np.ascontiguousarray(p2d), "g": np.ascontiguousarray(g2d)}
        )
    return in_maps


def _finish(results, p: np.ndarray):
    """Small all-reduce over the 250-lag statistics, in float64."""
    G = np.zeros((ROWS, NS), np.float64)
    PP = np.zeros((ROWS, 128), np.float64)
    S_p = S_g = Q_g = 0.0
    for r in results:
        o1 = r["out1"].astype(np.float64)
        G += o1[:, :NS]
        PP += o1[:, NS:OUT1C]
        o2 = r["out2"].astype(np.float64)
        S_p += o2[:, 0].sum()
        S_g += o2[:, 1].sum()
        Q_g += o2[:, 2].sum()
    Q_p = np.trace(PP)

    X = np.array([np.trace(G, offset=n) for n in range(NLAGS)])

    p64 = p.astype(np.float64)
    tail = p64[T - NLAGS + 1 :][::-1]  # last 249 elements, reversed
    R = np.concatenate([[0.0], np.cumsum(tail)])        # R[n], n=0..249
    R2 = np.concatenate([[0.0], np.cumsum(tail * tail)])

    m = S_g / T
    var_g = (Q_g - T * m * m) / (T - 1)
    Sv = S_g - T * m

    sum_n = S_p - R
    mp = sum_n / T
    sumsq_n = Q_p - R2
    var_p = (sumsq_n - T * mp * mp) / (T - 1)
    cov = (X - m * sum_n - mp * Sv) / T
    denom = var_g + var_p + (m - mp) ** 2
    ccc = 2.0 * cov / denom
    return np.float32(1.0 - ccc.mean())


def kernel(prediction: np.ndarray, ground_truth: np.ndarray) -> np.ndarray:
    from concourse import bass_utils

    p = np.asarray(prediction, np.float32).reshape(-1)
    g = np.asarray(ground_truth, np.float32).reshape(-1)
    assert p.shape == (T,) and g.shape == (T,)

    nc = _get_compiled()
    in_maps = _shard_inputs(p, g)
    res = bass_utils.run_bass_kernel_spmd(nc, in_maps, core_ids=list(range(N_CORES)))
    return _finish(res.results, p)


# revision 4
# speedup vs baseline: 1.2538x; 1.0056x over previous
"""CrossCCC loss kernel for Trainium2 (8 NeuronCores, sequence-parallel).

Math
----
reference computes, for lags n = 0..249:
    pred_n = [n zeros] ++ prediction[:T-n]
    ccc_n  = 2*cov(pred_n, gt) / (var_gt + var_pred_n + (mean_gt - mean_pred_n)^2)
    out    = 1 - mean_n(ccc_n)

Every lag statistic decomposes into lag-independent global sums plus tiny
suffix corrections:
    sum_n   = S_p - R_n          (R_n = sum of last n elements of p)
    sumsq_n = Q_p - R2_n
    cov_n   = (X_n - mean_gt*sum_n - mean_pred_n*Sv) / T,   Sv = S_g - T*mean_gt
with X_n = sum_j p[j]*gt[j+n] the raw cross-correlation -- the only heavy
term.  With j = 128*a + k:
    X_n = sum_k G[k, k+n],   G[k, s] = sum_a p[128a + k] * gt[128a + s]
for s in [0, 384): a Gram-style matmul contracting over the long block axis.
Q_p likewise falls out of trace(PP), PP[k,k'] = sum_a p[128a+k]*p[128a+k'].

Sharding: blocks are split across 8 cores.  Each core views its
131072-element p shard as [128, 1024] row-major (row q covers blocks
8q..8q+7) and its gt shard (+256 halo) as overlapping rows [128, 1280], so
for column-tile t in 0..7:
    G  += p2d[:, 128t:128t+128].T @ gt2d[:, 128t:128t+384]   (PSUM accumulate)
    PP += p2d[:, 128t:128t+128].T @ p2d[:, 128t:128t+128]
Inputs are pre-cast to bf16 on the host (exact bf16 products accumulate in
fp32 PSUM; final result error ~1e-6 relative).  Per-core partial sums of p,
g, g^2 ride along on the vector/scalar engines.  Host sums the 8 partial
G/PP's, takes diagonal traces, and finishes the scalar formula in float64.
"""

import numpy as np

T = 1_000_000
N_CORES = 8
ROWS = 128          # SBUF partitions; also the k-lane count
COLS = 1024         # per-row elements = 8 column-tiles of 128
SHARD = ROWS * COLS  # 131072 elements of p per core
HALO = 256           # gt halo: max lag reach 249 rounded to 2 blocks
GCOLS = COLS + HALO  # 1280
NTILES = COLS // 128  # 8
NS = 384             # G free size: covers s = k + n, n<250, k<128
NLAGS = 250
OUT1C = NS + 128     # bf16 matmul results: [G | PP]
OUT2C = 3            # f32 partial sums: [sum_p | sum_g | sumsq_g]

_compiled = None


def _build():
    import concourse.bacc as bacc
    import concourse.mybir as mybir
    import concourse.tile as tile

    f32 = mybir.dt.float32
    bf16 = mybir.dt.bfloat16
    nc = bacc.Bacc("TRN2", target_bir_lowering=False, debug=False)

    p_dram = nc.dram_tensor("p", [ROWS, COLS], bf16, kind="ExternalInput")
    g_dram = nc.dram_tensor("g", [ROWS, GCOLS], bf16, kind="ExternalInput")
    out1_dram = nc.dram_tensor("out1", [ROWS, OUT1C], bf16, kind="ExternalOutput")
    out2_dram = nc.dram_tensor("out2", [ROWS, OUT2C], f32, kind="ExternalOutput")

    with tile.TileContext(nc) as tc:
        with (
            tc.tile_pool(name="io", bufs=1) as io_pool,
            tc.tile_pool(name="scratch", bufs=1) as scratch_pool,
            tc.tile_pool(name="psum", bufs=1, space="PSUM") as psum_pool,
        ):
            pb = io_pool.tile([ROWS, COLS], bf16)
            gb = io_pool.tile([ROWS, GCOLS], bf16)
            out1 = io_pool.tile([ROWS, OUT1C], bf16)
            out2 = io_pool.tile([ROWS, OUT2C], f32)

            # PE warmup: dummy matmuls with no input deps keep the tensor
            # engine busy during the input-DMA wait so the HAM clock gate
            # opens (1.2 -> 2.4 GHz) before the real matmuls run.
            warm = scratch_pool.tile([ROWS, 512], bf16, tag="warm")
            nc.gpsimd.memset(warm[:], 0.0)
            wsum = psum_pool.tile([ROWS, 512], f32, tag="wsum")
            for _ in range(12):
                nc.tensor.matmul(wsum[:], warm[:, :128], warm[:])

            # split loads across both HWDGE rings so tile-0 operands land early
            nc.scalar.dma_start(pb[:], p_dram[:])
            nc.sync.dma_start(gb[:, 0:512], g_dram[:, 0:512])
            nc.sync.dma_start(gb[:, 512:GCOLS], g_dram[:, 512:GCOLS])

            gram = psum_pool.tile([ROWS, NS], f32)
            pp = psum_pool.tile([ROWS, 128], f32)
            for t in range(NTILES):
                lhsT = pb[:, t * 128 : t * 128 + 128]
                nc.tensor.matmul(
                    pp[:],
                    lhsT,
                    pb[:, t * 128 : t * 128 + 128],
                    start=(t == 0),
                    stop=(t == NTILES - 1),
                )
                nc.tensor.matmul(
                    gram[:],
                    lhsT,
                    gb[:, t * 128 : t * 128 + NS],
                    start=(t == 0),
                    stop=(t == NTILES - 1),
                )

            # per-partition scalar partials
            sq = scratch_pool.tile([ROWS, COLS], bf16)
            nc.vector.reduce_sum(out2[:, 0:1], pb[:], axis=mybir.AxisListType.X)
            nc.vector.reduce_sum(
                out2[:, 1:2], gb[:, :COLS], axis=mybir.AxisListType.X
            )
            nc.scalar.activation(
                sq[:],
                gb[:, :COLS],
                mybir.ActivationFunctionType.Square,
                accum_out=out2[:, 2:3],
            )

            nc.scalar.copy(out1[:, NS:OUT1C], pp[:])
            nc.vector.tensor_copy(out1[:, :NS], gram[:])
            nc.scalar.dma_start(out2_dram[:], out2[:])
            nc.sync.dma_start(out1_dram[:], out1[:])

    nc.compile()
    return nc


def _get_compiled():
    global _compiled
    if _compiled is None:
        _compiled = _build()
    return _compiled


def _shard_inputs(p: np.ndarray, g: np.ndarray):
    import ml_dtypes

    bf = ml_dtypes.bfloat16
    p_pad = np.zeros(N_CORES * SHARD, bf)
    p_pad[:T] = p.astype(bf)
    g_pad = np.zeros(N_CORES * SHARD + HALO, bf)
    g_pad[:T] = g.astype(bf)
    in_maps = []
    for c in range(N_CORES):
        p2d = p_pad[c * SHARD : (c + 1) * SHARD].reshape(ROWS, COLS)
        base = g_pad[c * SHARD : (c + 1) * SHARD + HALO]
        g2d = np.lib.stride_tricks.as_strided(
            base, shape=(ROWS, GCOLS), strides=(2 * COLS, 2)
        )
        in_maps.append(
            {"p": np.ascontiguousarray(p2d), "g": np.ascontiguousarray(g2d)}
        )
    return in_maps


def _finish(results, p: np.ndarray):
    """Small all-reduce over the 250-lag statistics, in float64."""
    G = np.zeros((ROWS, NS), np.float64)
    PP = np.zeros((ROWS, 128), np.float64)
    S_p = S_g = Q_g = 0.0
    for r in results:
        o1 = r["out1"].astype(np.float64)
        G += o1[:, :NS]
        PP += o1[:, NS:OUT1C]
        o2 = r["out2"].astype(np.float64)
        S_p += o2[:, 0].sum()
        S_g += o2[:, 1].sum()
        Q_g += o2[:, 2].sum()
    Q_p = np.trace(PP)

    X = np.array([np.trace(G, offset=n) for n in range(NLAGS)])

    p64 = p.astype(np.float64)
    tail = p64[T - NLAGS + 1 :][::-1]  # last 249 elements, reversed
    R = np.concatenate([[0.0], np.cumsum(tail)])        # R[n], n=0..249
    R2 = np.concatenate([[0.0], np.cumsum(tail * tail)])

    m = S_g / T
    var_g = (Q_g - T * m * m) / (T - 1)
    Sv = S_g - T * m

    sum_n = S_p - R
    mp = sum_n / T
    sumsq_n = Q_p - R2
    var_p = (sumsq_n - T * mp * mp) / (T - 1)
    cov = (X - m * sum_n - mp * Sv) / T
    denom = var_g + var_p + (m - mp) ** 2
    ccc = 2.0 * cov / denom
    return np.float32(1.0 - ccc.mean())


def kernel(prediction: np.ndarray, ground_truth: np.ndarray) -> np.ndarray:
    from concourse import bass_utils

    p = np.asarray(prediction, np.float32).reshape(-1)
    g = np.asarray(ground_truth, np.float32).reshape(-1)
    assert p.shape == (T,) and g.shape == (T,)

    nc = _get_compiled()
    in_maps = _shard_inputs(p, g)
    res = bass_utils.run_bass_kernel_spmd(nc, in_maps, core_ids=list(range(N_CORES)))
    return _finish(res.results, p)
